# revision 13
# baseline (speedup 1.0000x reference)
"""Causal attention (B=8, S=2048, D=1024, fp32) on 8 TRN2 NeuronCores.

Sharding: batch-parallel, one batch element per core (SPMD, no collectives).

Per-core algorithm (S^T layout):
  - Q, K are loaded with an fp32->bf16 cast during DMA, then transposed on
    TensorE into [d, s] layouts QT/KT. Transposes are PLAIN matmuls against a
    bf16 identity moving operand (out = X^T @ I): the stationary load then
    uses Fast Weight Load (2x), so a 128x128 transpose costs ~53ns instead of
    the ~107ns of transpose-mode (whose LDWEIGHTS gets no FWL).
  - Scores are computed transposed: S^T[k, q] = sum_d KT[d,k] * QT[d,q],
    accumulated over 8 d-subtiles in PSUM, 2 k-tiles x 256 q per PSUM bank.
  - Causal mask: multiplicative bf16 0/1 mask applied to P^T on the diagonal
    pair only; k-tiles above the diagonal are skipped entirely.
  - exp((dots)/sqrt(D)) on ScalarE (no max subtraction: |dots| <= ~1.1e3 so
    logits <= ~35, exp fits fp32 comfortably), output cast to bf16 = P^T.
  - PV: O[q, d] += P^T.T @ V with V in native [k, d] layout; row sums via an
    extra N=1 matmul against a ones vector; final normalization is a DVE
    multiply by the reciprocal row sum.
  - Software pipelining: the QK matmuls for pair p+1 (or the next group's
    transposes, on the last pair) are emitted BETWEEN exp(p) and PV(p), so
    the PE never sits waiting on ScalarE's exp.
  - DMA: loads are issued before mask setup and spread across three engine
    queues (Q on GpSimd, K on Scalar, V on DVE); 3 groups are prefetched up
    front to keep HBM busy through the compute-light early groups.
"""

import numpy as np

import concourse.bass as bass
import concourse.mybir as mybir
import concourse.tile as tile
from concourse import bacc
from concourse.masks import make_identity

P = 128


def build_attention_nc(S=2048, D=1024):
    f32, bf16 = mybir.dt.float32, mybir.dt.bfloat16
    nc = bacc.Bacc(None, target_bir_lowering=False)

    q_d = nc.dram_tensor("query", [S, D], f32, kind="ExternalInput")
    k_d = nc.dram_tensor("key", [S, D], f32, kind="ExternalInput")
    v_d = nc.dram_tensor("value", [S, D], f32, kind="ExternalInput")
    o_d = nc.dram_tensor("out", [S, D], f32, kind="ExternalOutput")

    NT = S // P            # number of 128-row seq tiles
    ND = D // P            # number of 128-wide d subtiles
    QGT = 2                # q-tiles per group
    QG = QGT * P           # q-group width (256)
    NG = S // QG           # number of q groups
    DH = min(D, 512)       # PV free-dim chunk (one PSUM bank)
    NDH = D // DH
    TCH = 4 if ND % 4 == 0 else (2 if ND % 2 == 0 else 1)  # transpose chunk
    PREF = 3               # groups of loads kept in flight
    scale = 1.0 / float(np.sqrt(D))

    qv = q_d.rearrange("(n p) d -> p n d", p=P)
    kv = k_d.rearrange("(n p) d -> p n d", p=P)
    vv = v_d.rearrange("(n p) d -> p n d", p=P)
    ov = o_d.rearrange("(n p) d -> p n d", p=P)

    with tile.TileContext(nc) as tc:
        with (
            tc.tile_pool(name="const", bufs=1) as constp,
            tc.tile_pool(name="slab", bufs=1) as slab,
            tc.tile_pool(name="stage", bufs=2 * PREF) as stagep,
            tc.tile_pool(name="pt", bufs=3) as ptp,
            tc.tile_pool(name="small", bufs=2) as smallp,
            tc.tile_pool(name="ost", bufs=2) as ostp,
            tc.tile_pool(name="ps", bufs=1, space="PSUM") as psp,
        ):
            # Tiny first DMA to absorb the ~3us software-dynamic DMA queue
            # spin-up while the framework preamble still runs.
            ringw = constp.tile([8, 512], bf16)
            nc.gpsimd.dma_start(ringw[:], qv[:8, 0, :512])

            # Warmup tile (all ones): the PE warmup matmuls gate only on this
            # memset, not on identity's affine_select, so they start early.
            warmz = constp.tile([P, P], bf16)
            nc.gpsimd.memset(warmz[:], 1.0)
            ident = constp.tile([P, P], bf16)
            make_identity(nc, ident[:])
            ones = constp.tile([P, 1], bf16)
            nc.vector.memset(ones[:], 1.0)

            QT = slab.tile([P, ND, S], bf16)   # [d%128, d//128, q]
            KT = slab.tile([P, ND, S], bf16)   # [d%128, d//128, k]
            V = slab.tile([P, NT, D], bf16)    # [k%128, k//128, d]

            stages = {}

            def emit_qk_loads(g, halves=False):
                """Issue cast-DMAs for group g's new Q/K tiles.

                Cast DMAs must issue from GpSimd; one DMA per tensor per
                group (both 128-row tiles at once, 8KB/line reads) keeps the
                engine-side issue cost (~800ns per dma_start) low. Group 0's
                Q/K are split in half-D chunks so the first transposes can
                start as soon as the first halves have landed.
                """
                t0 = QGT * g
                stq = stagep.tile([P, QGT, D], bf16, tag="stage", name=f"stg_q{g}")
                stk = stagep.tile([P, QGT, D], bf16, tag="stage", name=f"stg_k{g}")
                for nm, stg, srcv in (("q", stq, qv), ("k", stk, kv)):
                    if halves:
                        hd = D // 2
                        nc.gpsimd.dma_start(
                            stg[:, :, :hd], srcv[:, t0 : t0 + QGT, :hd]
                        )
                        nc.gpsimd.dma_start(
                            stg[:, :, hd:], srcv[:, t0 : t0 + QGT, hd:]
                        )
                    else:
                        nc.gpsimd.dma_start(stg[:], srcv[:, t0 : t0 + QGT, :])
                    stages[(nm, g)] = stg

            def emit_v_load(g):
                # V(g) is first read at PV of group g's diagonal pair, which
                # runs AFTER the transposes consuming q/k(g+1) — issue it
                # after those on the (in-order) DMA queue.
                t0 = QGT * g
                nc.gpsimd.dma_start(
                    V[:, t0 : t0 + QGT, :], vv[:, t0 : t0 + QGT, :]
                )  # fp32->bf16

            emit_qk_loads(0, halves=True)

            # Multiplicative causal mask for the diagonal k-tile pair, S^T
            # layout (1=valid, 0=masked), applied to P^T after exp:
            # mask01[kk, half, qq] = 1 if (128*half + kk) <= qq else 0.
            # Emitted after group 0's Q loads so DMA starts first.
            mask01 = constp.tile([P, 2, QG], bf16)
            for half in range(2):
                m = mask01[:, half, :]
                nc.gpsimd.memset(m, 1.0)
                nc.gpsimd.affine_select(
                    out=m,
                    in_=m,
                    compare_op=mybir.AluOpType.is_ge,
                    fill=0.0,
                    base=-(P * half),
                    pattern=[[1, QG]],
                    channel_multiplier=-1,
                )

            # Remaining prologue loads in need order: q/k(g+1) are consumed
            # (by transposes) BEFORE v(g) (by the diagonal PV).
            for g in range(1, min(PREF, NG)):
                emit_qk_loads(g)
                emit_v_load(g - 1)

            # Warm the PE clock gate (HAM) during the DMA-bound startup:
            # bf16 matmuls on the ones tile depend only on its memset and
            # give ~2us of busy PE time right before the first transposes.
            warm = psp.tile([P, P], f32, tag="st", bufs=3)
            for _ in range(32):
                nc.tensor.matmul(
                    warm[:], lhsT=warmz[:], rhs=warmz[:],
                    start=True, stop=True,
                )

            def emit_transposes(g):
                # Q tiles first: group g's QK matmuls need QT immediately,
                # but the new KT tiles only at the diagonal (last) pair.
                for nm, dst in (("q", QT), ("k", KT)):
                    stg = stages.pop((nm, g))
                    for tt in range(QGT):
                        t = QGT * g + tt
                        for c in range(ND // TCH):
                            pst = psp.tile([P, TCH, P], f32, tag="st", bufs=3)
                            for j in range(TCH):
                                ds = c * TCH + j
                                nc.tensor.matmul(
                                    pst[:, j, :],
                                    lhsT=stg[:, tt, ds * P : (ds + 1) * P],
                                    rhs=ident[:],
                                    start=True,
                                    stop=True,
                                )
                            dslc = dst[:, c * TCH : (c + 1) * TCH, t * P : (t + 1) * P]
                            if nm == "q":
                                nc.vector.tensor_copy(dslc, pst[:])
                            else:
                                nc.scalar.copy(dslc, pst[:])

            def emit_qk(g, p):
                """QK matmuls for pair p of group g into a fresh stps tile."""
                diag = p == g
                stps = psp.tile([P, 2, QG], f32, tag="st", bufs=3)
                for kk in range(2):
                    ki = 2 * p + kk
                    # Diagonal pair, second k-tile: q < 128 (rel) is fully
                    # masked, so only compute the upper q half (N=128).
                    qlo = P if (diag and kk == 1) else 0
                    for ds in range(ND):
                        nc.tensor.matmul(
                            stps[:, kk, qlo:],
                            lhsT=KT[:, ds, ki * P : (ki + 1) * P],
                            rhs=QT[:, ds, g * QG + qlo : (g + 1) * QG],
                            start=(ds == 0),
                            stop=(ds == ND - 1),
                        )
                if diag:
                    # The uncomputed quarter holds stale PSUM garbage: give it
                    # a finite value; the multiplicative mask below zeroes it
                    # (and all other masked entries) after exp.
                    nc.vector.memset(stps[:, 1, :P], 0.0)
                return stps

            def emit_filler(rsw, n):
                """Dummy matmuls that keep the PE executing (and the HAM
                clock gate at full speed) across an expected DMA-wait gap.
                They accumulate garbage into a scratch region of the rsw
                bank; start=False so the row-sum accumulation's has_written
                state is never touched."""
                for _ in range(n):
                    nc.tensor.matmul(
                        rsw[:, 64:192],
                        lhsT=warmz[:],
                        rhs=warmz[:],
                        start=False,
                        stop=False,
                        skip_group_check=True,
                    )

            emit_transposes(0)
            for g in range(NG):
                if g + PREF < NG:
                    emit_qk_loads(g + PREF)
                    emit_v_load(g + PREF - 1)
                elif g + PREF == NG:
                    emit_v_load(NG - 1)

                # One PSUM tile per (q-tile, d-half) so each bank is released
                # as soon as its own normalize-read completes.
                opv = [
                    [
                        psp.tile(
                            [P, DH], f32, tag=f"pv{j}_{dh}", bufs=1,
                            name=f"opv{j}_{dh}",
                        )
                        for dh in range(NDH)
                    ]
                    for j in range(QGT)
                ]
                # One bank: row sums in [:, :QGT], filler scratch in [64:192].
                rsw = psp.tile([P, 192], f32, tag="rs", bufs=1)

                stps = emit_qk(g, 0)
                for p in range(g + 1):
                    diag = p == g
                    ptt = ptp.tile([P, 2, QG], bf16, tag="pt")
                    nc.scalar.activation(
                        ptt[:], stps[:], mybir.ActivationFunctionType.Exp,
                        scale=scale,
                    )
                    # Mask-mul before the transpose copies so it isn't queued
                    # behind them on DVE (PV waits on it).
                    if diag:
                        nc.vector.tensor_mul(ptt[:], ptt[:], mask01[:])
                    # Fill the PE while ScalarE runs exp(p): either the next
                    # pair's QK matmuls or the next group's transposes.
                    if p < g:
                        stps = emit_qk(g, p + 1)
                    elif g + 1 < NG:
                        # Early groups outrun the HBM stream: burn the
                        # expected wait on filler so HAM stays at full speed.
                        emit_filler(rsw, max(0, 40 - 12 * g))
                        emit_transposes(g + 1)
                        emit_filler(rsw, 8 if g <= 4 else 0)
                    for kk in range(2):
                        ki = 2 * p + kk
                        first = (p == 0) and (kk == 0)
                        for j in range(QGT):
                            if diag and kk == 1 and j == 0:
                                continue  # fully masked block
                            # last matmul touching opv[j]'s accumulation:
                            last_j = diag and (kk == 1 or (kk == 0 and j == 0))
                            lh = ptt[:, kk, j * P : (j + 1) * P]
                            for dh in range(NDH):
                                nc.tensor.matmul(
                                    opv[j][dh][:],
                                    lhsT=lh,
                                    rhs=V[:, ki, dh * DH : (dh + 1) * DH],
                                    start=first,
                                    stop=last_j,
                                )
                            # rsps is one PSUM bank = one zero region: start
                            # exactly once (marks whole bank pending-zero, so
                            # each column's first write lands as overwrite).
                            nc.tensor.matmul(
                                rsw[:, j : j + 1],
                                lhsT=lh,
                                rhs=ones[:],
                                start=(first and j == 0),
                                stop=(diag and kk == 1 and j == QGT - 1),
                            )

                # ---- normalize + store (per d-half, shipping each half as
                # soon as it is scaled; final group splits across DVE+ACT
                # since no later exp can be delayed) ----
                rec = smallp.tile([P, QGT], f32, tag="rec")
                nc.vector.reciprocal(rec[:], rsw[:, :QGT])
                final = g == NG - 1
                for j in range(QGT):
                    ost = ostp.tile([P, D], f32, tag="ost")
                    for dh in range(NDH):
                        osl = ost[:, dh * DH : (dh + 1) * DH]
                        if final and dh % 2 == 1:
                            nc.scalar.mul(osl, opv[j][dh][:], mul=rec[:, j : j + 1])
                        else:
                            nc.vector.tensor_scalar_mul(
                                osl, opv[j][dh][:], scalar1=rec[:, j : j + 1]
                            )
                        nc.sync.dma_start(
                            ov[:, g * QGT + j, dh * DH : (dh + 1) * DH], osl
                        )

    nc.compile()
    return nc


_NC_CACHE = {}


def _get_nc(S, D):
    if (S, D) not in _NC_CACHE:
        _NC_CACHE[(S, D)] = build_attention_nc(S, D)
    return _NC_CACHE[(S, D)]


def kernel(query, key, value):
    from concourse.bass_utils import run_bass_kernel_spmd

    query = np.asarray(query, dtype=np.float32)
    key = np.asarray(key, dtype=np.float32)
    value = np.asarray(value, dtype=np.float32)
    B, S, D = query.shape
    nc = _get_nc(S, D)
    in_maps = [
        {
            "query": np.ascontiguousarray(query[i]),
            "key": np.ascontiguousarray(key[i]),
            "value": np.ascontiguousarray(value[i]),
        }
        for i in range(B)
    ]
    res = run_bass_kernel_spmd(nc, in_maps, core_ids=list(range(B)))
    out = np.stack([r["out"] for r in res.results], axis=0)
    return out.astype(np.float32)
